# revision 18
# baseline (speedup 1.0000x reference)
import math
import numpy as np
import ml_dtypes
from contextlib import ExitStack

import concourse.bass as bass
import concourse.tile as tile
from concourse import bacc, mybir
from concourse.bass_utils import run_bass_kernel_spmd

F32 = mybir.dt.float32
BF16 = mybir.dt.bfloat16
I16 = mybir.dt.int16
I32 = mybir.dt.int32


class Cfg:
    def __init__(self, N, E, F=128, HID=128, C=32, K=3, NCORE=8, CHUNK=None,
                 GBLK=32):
        self.N, self.E, self.F, self.HID, self.C, self.K = N, E, F, HID, C, K
        self.NCORE = NCORE
        if CHUNK is None:
            CHUNK = ((N + NCORE - 1) // NCORE + 127) // 128 * 128
        self.CHUNK = CHUNK
        self.NPAD = NCORE * CHUNK
        self.TILES = CHUNK // 128
        self.STRIPES = self.NPAD // 128
        self.GBLK = GBLK
        self.TA = (self.TILES + 1) // 2
        self.TB = self.TILES - self.TA
        self.RA = self.TA * 128
        self.RB = self.TB * 128
        self.HALF0 = NCORE * self.RA
        self.HALF1 = NCORE * self.RB
        assert self.HALF0 <= 32768 and self.HALF1 <= 32768, "int16 idx limit"

    def rowmap(self, n):
        c = n // self.CHUNK
        o = n % self.CHUNK
        return np.where(o < self.RA, c * self.RA + o,
                        self.HALF0 + c * self.RB + (o - self.RA))


def preprocess(cfg, edge_index, deg):
    c = cfg
    src, dst = edge_index[0].astype(np.int64), edge_index[1].astype(np.int64)
    owner = dst // c.CHUNK
    t_all = (dst % c.CHUNK) >> 7
    p_all = dst & 127
    srow = cfg.rowmap(src)
    h_all = (srow >= c.HALF0).astype(np.int64)

    key = (owner * c.TILES + t_all) * 2 + h_all
    order = np.lexsort((srow, key))
    key_s = key[order]
    srow_s = srow[order]
    p_s = p_all[order]
    ngrp = c.NCORE * c.TILES * 2
    counts = np.bincount(key_s, minlength=ngrp).reshape(c.NCORE, c.TILES, 2)
    starts = np.zeros(ngrp + 1, np.int64)
    np.cumsum(counts.reshape(-1), out=starts[1:])

    nblk = (counts + 127) // 128
    B = nblk.max(axis=0)
    B0, B1 = B[:, 0].copy(), B[:, 1].copy()
    NB0, NB1 = int(B0.sum()), int(B1.sum())
    TOTBLK = NB0 + NB1

    idx0 = np.zeros((c.NCORE, NB0 * 128), np.int16)
    idx1 = np.zeros((c.NCORE, NB1 * 128), np.int16)
    pv = np.full((c.NCORE, TOTBLK * 128), 128.0, np.float32)

    s0_off = np.concatenate([[0], np.cumsum(B0)])
    s1_off = np.concatenate([[0], np.cumsum(B1)])

    for core in range(c.NCORE):
        for t in range(c.TILES):
            for h, (idxa, soff) in enumerate(((idx0, s0_off), (idx1, s1_off))):
                g = (core * c.TILES + t) * 2 + h
                n = int(counts[core, t, h])
                a = int(starts[g])
                lidx = (srow_s[a : a + n] - h * c.HALF0).astype(np.int16)
                ps = p_s[a : a + n].astype(np.float32)
                base = int(soff[t]) * 128
                idxa[core, base : base + n] = lidx
                pbase = (0 if h == 0 else NB0 * 128) + base
                pv[core, pbase : pbase + n] = ps

    def wrap_idx(a):
        m = a.reshape(a.shape[0], -1, 16)
        m = np.swapaxes(m, 1, 2)
        return np.tile(m, (1, 8, 1)).copy()

    return dict(
        B0=B0, B1=B1, NB0=NB0, NB1=NB1, TOTBLK=TOTBLK,
        idx0=wrap_idx(idx0) if NB0 else np.zeros((c.NCORE, 128, 0), np.int16),
        idx1=wrap_idx(idx1) if NB1 else np.zeros((c.NCORE, 128, 0), np.int16),
        pv=pv.reshape(c.NCORE, TOTBLK, 128).transpose(0, 2, 1).copy(),
    )


def _calls(total, gblk):
    out = []
    b = 0
    while b < total:
        nb = min(gblk, total - b)
        out.append((b, nb))
        b += nb
    return out


def build_nc(cfg, meta):
    c = cfg
    B0, B1 = meta["B0"], meta["B1"]
    NB0, NB1, TOTBLK = meta["NB0"], meta["NB1"], meta["TOTBLK"]
    calls0 = _calls(NB0, c.GBLK)
    calls1 = _calls(NB1, c.GBLK)
    s0_off = np.concatenate([[0], np.cumsum(B0)])
    s1_off = np.concatenate([[0], np.cumsum(B1)])

    nc = bacc.Bacc(None, target_bir_lowering=False, num_swdge_queues=4)

    xfull = nc.declare_dram_parameter("xfull", [128, c.STRIPES, c.F], F32, isOutput=False)
    xchunk = nc.declare_dram_parameter("xchunk", [128, c.TILES, c.F], F32, isOutput=False)
    idx0_d = nc.declare_dram_parameter("idx0", [128, max(NB0 * 8, 1)], I16, isOutput=False)
    idx1_d = nc.declare_dram_parameter("idx1", [128, max(NB1 * 8, 1)], I16, isOutput=False)
    m_d = nc.declare_dram_parameter("m_in", [128, TOTBLK, 128], BF16, isOutput=False)
    discol_d = nc.declare_dram_parameter("discol", [128, c.TILES], F32, isOutput=False)
    disall_d = nc.declare_dram_parameter("disall", [128, c.STRIPES], F32, isOutput=False)
    w1_d = nc.declare_dram_parameter("w1", [c.K + 1, c.F, c.HID], F32, isOutput=False)
    b1_d = nc.declare_dram_parameter("b1", [c.HID, 1], F32, isOutput=False)
    w2_d = nc.declare_dram_parameter("w2", [c.K + 1, c.HID, c.C], F32, isOutput=False)
    b2_d = nc.declare_dram_parameter("b2", [c.C, 1], F32, isOutput=False)
    out_d = nc.declare_dram_parameter("out", [c.CHUNK, c.C], F32, isOutput=True)

    ta = [nc.dram_tensor(f"tab_a{i}", [c.HALF0, c.F], BF16, kind="Internal",
                         addr_space="Shared") for i in range(2)]
    tb = [nc.dram_tensor(f"tab_b{i}", [c.HALF1, c.F], BF16, kind="Internal",
                         addr_space="Shared") for i in range(2)]
    stage_a = nc.dram_tensor("stage_a", [c.RA, c.F], BF16, kind="Internal")
    stage_b = nc.dram_tensor("stage_b", [c.RB, c.F], BF16, kind="Internal")

    with tile.TileContext(nc) as tc, ExitStack() as ctx:
        sp = ctx.enter_context(tc.tile_pool(name="sp", bufs=1))
        gp0 = ctx.enter_context(tc.tile_pool(name="gp0", bufs=4))
        gp1 = ctx.enter_context(tc.tile_pool(name="gp1", bufs=4))
        mp = ctx.enter_context(tc.tile_pool(name="mp", bufs=3))
        wp = ctx.enter_context(tc.tile_pool(name="wp", bufs=2))
        ps_seg = ctx.enter_context(tc.tile_pool(name="ps_seg", bufs=4, space="PSUM"))
        ps_tr = ctx.enter_context(tc.tile_pool(name="ps_tr", bufs=2, space="PSUM"))
        ps_w = ctx.enter_context(tc.tile_pool(name="ps_w", bufs=2, space="PSUM"))

        idx0_t = sp.tile([128, max(NB0 * 8, 1)], I16)
        nc.sync.dma_start(idx0_t[:], idx0_d[:, :])
        idx1_t = sp.tile([128, max(NB1 * 8, 1)], I16)
        nc.sync.dma_start(idx1_t[:], idx1_d[:, :])
        discol_t = sp.tile([128, c.TILES], F32)
        nc.sync.dma_start(discol_t[:], discol_d[:, :])
        disall_t = sp.tile([128, c.STRIPES], F32)
        nc.sync.dma_start(disall_t[:], disall_d[:, :])
        b1_t = sp.tile([c.HID, 1], F32)
        nc.sync.dma_start(b1_t[:], b1_d[:, :])
        b2_t = sp.tile([c.C, 1], F32)
        nc.sync.dma_start(b2_t[:], b2_d[:, :])

        w1_t, w2_t = [], []
        for k in range(c.K + 1):
            wf = wp.tile([c.F, c.HID], F32, tag="wload", name="wload")
            nc.sync.dma_start(wf[:], w1_d[k, :, :])
            wb = sp.tile([c.F, c.HID], BF16, tag=f"w1_{k}", name=f"w1_{k}")
            nc.vector.tensor_copy(wb[:], wf[:])
            w1_t.append(wb)
        for k in range(c.K + 1):
            wf = wp.tile([c.HID, c.C], F32, tag="wload2", name="wload2")
            nc.sync.dma_start(wf[:], w2_d[k, :, :])
            wb = sp.tile([c.HID, c.C], BF16, tag=f"w2_{k}", name=f"w2_{k}")
            nc.vector.tensor_copy(wb[:], wf[:])
            w2_t.append(wb)

        iota_i = sp.tile([128, 128], I32)
        nc.gpsimd.iota(iota_i[:], pattern=[[1, 128]], base=0, channel_multiplier=0)
        iota_bf = sp.tile([128, 128], BF16)
        nc.vector.tensor_copy(iota_bf[:], iota_i[:])
        iota_f = sp.tile([128, 128], F32)
        nc.vector.tensor_copy(iota_f[:], iota_i[:])
        iop_i = sp.tile([128, 1], I32)
        nc.gpsimd.iota(iop_i[:], pattern=[[1, 1]], base=0, channel_multiplier=1)
        iop_f = sp.tile([128, 1], F32)
        nc.vector.tensor_copy(iop_f[:], iop_i[:])
        ident_bf = sp.tile([128, 128], BF16)
        nc.vector.tensor_scalar(ident_bf[:], iota_bf[:], iop_f[:], None,
                                mybir.AluOpType.is_equal)
        ident_f = sp.tile([128, 128], F32)
        nc.vector.tensor_scalar(ident_f[:], iota_f[:], iop_f[:], None,
                                mybir.AluOpType.is_equal)

        stash = [sp.tile([128, c.CHUNK], BF16, tag=f"stash{k}", name=f"stash{k}")
                 for k in range(c.K + 1)]
        staging = sp.tile([128, c.TILES, c.F], BF16, tag="staging", name="staging")

        gfull_reg = nc.gpsimd.to_reg(c.GBLK * 128)

        SB = 2
        SA = c.HALF0 // 128
        for s in range(0, c.STRIPES, SB):
            n = min(SB, c.STRIPES - s)
            xt = wp.tile([128, SB, c.F], F32, tag="g0x", name="g0x")
            nc.sync.dma_start(xt[:, 0:n, :], xfull[:, s : s + n, :])
            gt = wp.tile([128, SB, c.F], BF16, tag="g0g", name="g0g")
            for j in range(n):
                if (s + j) % 2 == 0:
                    nc.vector.tensor_scalar(gt[:, j, :], xt[:, j, :],
                                            disall_t[:, s + j : s + j + 1], None,
                                            mybir.AluOpType.mult)
                else:
                    nc.scalar.activation(gt[:, j, :], xt[:, j, :],
                                         mybir.ActivationFunctionType.Copy,
                                         scale=disall_t[:, s + j : s + j + 1])
            if s + n <= SA:
                nc.sync.dma_start(
                    ta[0][s * 128 : (s + n) * 128, :].rearrange(
                        "(j p) f -> p j f", p=128), gt[:, 0:n, :])
            elif s >= SA:
                nc.sync.dma_start(
                    tb[0][(s - SA) * 128 : (s - SA + n) * 128, :].rearrange(
                        "(j p) f -> p j f", p=128), gt[:, 0:n, :])
            else:
                nc.sync.dma_start(
                    ta[0][s * 128 : (s + 1) * 128, :].rearrange(
                        "(j p) f -> p j f", p=128), gt[:, 0:1, :])
                nc.sync.dma_start(
                    tb[0][0:128, :].rearrange(
                        "(j p) f -> p j f", p=128), gt[:, 1:2, :])

        for t in range(c.TILES):
            xt = wp.tile([128, c.F], F32, tag="xT", name="xT")
            nc.sync.dma_start(xt[:], xchunk[:, t, :])
            pt = ps_tr.tile([128, 128], F32, tag="ptr", name="ptr_f")
            nc.tensor.transpose(pt[:], xt[:], ident_f[:])
            nc.vector.tensor_copy(stash[0][:, t * 128 : (t + 1) * 128], pt[:])

        def ag_a(par_out):
            nc.sync.dma_start(
                stage_a[:, :].rearrange("(t p) f -> p t f", p=128),
                staging[:, 0 : c.TA, :])
            nc.gpsimd.collective_compute(
                "AllGather", mybir.AluOpType.bypass,
                replica_groups=[list(range(c.NCORE))],
                ins=[stage_a[:, :].opt()],
                outs=[ta[par_out][:, :].opt()],
            )

        def ag_b(par_out):
            nc.sync.dma_start(
                stage_b[:, :].rearrange("(t p) f -> p t f", p=128),
                staging[:, c.TA : c.TILES, :])
            nc.gpsimd.collective_compute(
                "AllGather", mybir.AluOpType.bypass,
                replica_groups=[list(range(c.NCORE))],
                ins=[stage_b[:, :].opt()],
                outs=[tb[par_out][:, :].opt()],
            )

        qctr = [0]

        def stream_slot(bufs, pos):
            for (b, nb, t_) in bufs:
                if b <= pos < b + nb:
                    return t_[:, pos - b, :]
            raise AssertionError(pos)

        def stage_tile(k, t):
            tr = slice(t * 128, (t + 1) * 128)
            pt = ps_tr.tile([128, 128], BF16, tag="ptr", name="ptr_b")
            nc.tensor.transpose(pt[:], stash[k][:, tr], ident_bf[:])
            nc.scalar.activation(staging[:, t, :], pt[:],
                                 mybir.ActivationFunctionType.Copy,
                                 scale=discol_t[:, t : t + 1])

        def hop(par, k, last):
            gatA, gatB, mA, mB = [], [], [], []

            def gcall(half, b, nb):
                pool, idxt, tab, moff = ((gp0, idx0_t, ta[par], 0) if half == 0
                                         else (gp1, idx1_t, tb[par], NB0))
                tag = f"g{half}buf"
                gt = pool.tile([128, c.GBLK, 128], BF16, tag=tag, name=tag)
                nreg = nb * 128
                nc.gpsimd.dma_gather(gt[:, 0:nb, :], tab[:, :],
                                     idxt[:, b * 8 : (b + nb) * 8],
                                     nb * 128, nreg, c.F,
                                     single_packet=False,
                                     queue_num=qctr[0] % 4)
                qctr[0] += 1
                (gatA if half == 0 else gatB).append((b, nb, gt))
                mt = mp.tile([128, c.GBLK, 128], BF16, tag="mstream", name="mstream")
                nc.sync.dma_start(mt[:, 0:nb, :], m_d[:, moff + b : moff + b + nb, :])
                (mA if half == 0 else mB).append((b, nb, mt))

            FRONT0, FRONT1 = 3, 2
            seq = [(0, b, nb) for (b, nb) in calls0[:FRONT0]]
            seq += [(1, b, nb) for (b, nb) in calls1[:FRONT1]]
            rest0, rest1 = calls0[FRONT0:], calls1[FRONT1:]
            i0 = i1 = 0
            while i1 < len(rest1) or i0 < len(rest0):
                if i0 < len(rest0):
                    seq.append((0,) + rest0[i0]); i0 += 1
                if i1 < len(rest1):
                    seq.append((1,) + rest1[i1]); i1 += 1
            for (half, b, nb) in seq:
                gcall(half, b, nb)

            for t in range(c.TILES):
                nb0, nb1 = int(B0[t]), int(B1[t])
                nbt = nb0 + nb1
                tr = slice(t * 128, (t + 1) * 128)
                if nbt == 0:
                    nc.vector.memset(stash[k][:, tr], 0.0)
                else:
                    ps = ps_seg.tile([128, 128], F32, tag="seg", name="seg")
                    j = 0
                    for jj in range(nb0):
                        bsl = stream_slot(gatA, int(s0_off[t]) + jj)
                        msl = stream_slot(mA, int(s0_off[t]) + jj)
                        nc.tensor.matmul(ps[:], bsl, msl, start=(j == 0),
                                         stop=(j == nbt - 1))
                        j += 1
                    for jj in range(nb1):
                        bsl = stream_slot(gatB, int(s1_off[t]) + jj)
                        msl = stream_slot(mB, int(s1_off[t]) + jj)
                        nc.tensor.matmul(ps[:], bsl, msl, start=(j == 0),
                                         stop=(j == nbt - 1))
                        j += 1
                    nc.vector.tensor_copy(stash[k][:, tr], ps[:])
                if not last:
                    stage_tile(k, t)
                    if t == c.TA - 1:
                        ag_a(1 - par)
            if not last:
                ag_b(1 - par)

        def layer_end(layer):
            if layer == 1:
                fout, w_t = c.HID, w1_t
            else:
                fout, w_t = c.C, w2_t
            for t in range(c.TILES):
                tr = slice(t * 128, (t + 1) * 128)
                ps = ps_w.tile([fout, 128], F32, tag="wps", name="wps")
                for k in range(c.K + 1):
                    nc.tensor.matmul(ps[:], w_t[k][:], stash[k][:, tr],
                                     start=(k == 0), stop=(k == c.K))
                if layer == 1:
                    nc.scalar.activation(stash[0][:, tr], ps[:],
                                         mybir.ActivationFunctionType.Relu,
                                         bias=b1_t[:, 0:1])
                    stage_tile(0, t)
                    if t == c.TA - 1:
                        ag_a(1)
                else:
                    o2 = wp.tile([c.C, 128], F32, tag="o2T", name="o2T")
                    nc.vector.tensor_scalar(o2[:], ps[:], b2_t[:, 0:1], None,
                                            mybir.AluOpType.add)
                    pt2 = ps_tr.tile([128, c.C], F32, tag="ptr", name="ptr_o")
                    nc.tensor.transpose(pt2[:], o2[:], ident_f[0 : c.C, 0 : c.C])
                    ot = wp.tile([128, c.C], F32, tag="ofin", name="ofin")
                    nc.vector.tensor_copy(ot[:], pt2[:])
                    nc.sync.dma_start(out_d[t * 128 : (t + 1) * 128, :], ot[:])
            if layer == 1:
                ag_b(1)

        hop(0, 1, False)
        hop(1, 2, False)
        hop(0, 3, True)
        layer_end(1)
        hop(1, 1, False)
        hop(0, 2, False)
        hop(1, 3, True)
        layer_end(2)

    nc.finalize()
    return nc


def make_host_data(cfg, inputs):
    c = cfg
    x = np.asarray(inputs["x"], np.float32)
    ei = np.asarray(inputs["edge_index"])
    w1 = np.asarray(inputs["w1"], np.float32)
    b1 = np.asarray(inputs["b1"], np.float32)
    w2 = np.asarray(inputs["w2"], np.float32)
    b2 = np.asarray(inputs["b2"], np.float32)

    deg = np.bincount(ei[1], minlength=c.N).astype(np.float32)
    dis = np.where(deg > 0, np.maximum(deg, 1.0) ** -0.5, 0.0).astype(np.float32)
    dis_pad = np.zeros(c.NPAD, np.float32)
    dis_pad[: c.N] = dis

    meta = preprocess(c, ei, deg)

    xpad = np.zeros((c.NPAD, c.F), np.float32)
    xpad[: c.N] = x
    rows = np.asarray(c.rowmap(np.arange(c.NPAD)))
    xtab = np.zeros_like(xpad)
    xtab[rows] = xpad
    distab = np.zeros(c.NPAD, np.float32)
    distab[rows] = dis_pad
    xfull = xtab.reshape(c.STRIPES, 128, c.F).transpose(1, 0, 2).copy()
    disall = distab.reshape(c.STRIPES, 128).T.copy()
    xchunk_all = xpad.reshape(c.NCORE, c.TILES, 128, c.F)

    B0, B1 = meta["B0"], meta["B1"]
    s0_off = np.concatenate([[0], np.cumsum(B0)])
    s1_off = np.concatenate([[0], np.cumsum(B1)])
    blk_tile = np.concatenate([
        np.repeat(np.arange(c.TILES), B0.astype(np.int64)),
        np.repeat(np.arange(c.TILES), B1.astype(np.int64)),
    ])

    in_maps = []
    for core in range(c.NCORE):
        r0, r1 = core * c.CHUNK, (core + 1) * c.CHUNK
        dchunk = dis_pad[r0:r1]
        pvc = meta["pv"][core]
        onehot = pvc[:, :, None] == np.arange(128, dtype=np.float32)[None, None, :]
        discols = dchunk.reshape(c.TILES, 128)[blk_tile]
        disb16 = discols.astype(ml_dtypes.bfloat16).astype(np.float32)
        m_in = (onehot * disb16[None, :, :]).astype(ml_dtypes.bfloat16)
        in_maps.append(dict(
            xfull=xfull,
            xchunk=xchunk_all[core].transpose(1, 0, 2).copy(),
            idx0=meta["idx0"][core],
            idx1=meta["idx1"][core],
            m_in=m_in,
            discol=dchunk.reshape(c.TILES, 128).T.copy(),
            disall=disall,
            w1=w1, b1=b1.reshape(c.HID, 1),
            w2=w2, b2=b2.reshape(c.C, 1),
        ))
    return meta, in_maps


def run(cfg, inputs, nc=None, meta=None, in_maps=None, trace=False):
    if meta is None or in_maps is None:
        meta, in_maps = make_host_data(cfg, inputs)
    if nc is None:
        nc = build_nc(cfg, meta)
    res = run_bass_kernel_spmd(nc, in_maps, list(range(cfg.NCORE)), trace=trace)
    outs = [res.results[i]["out"] for i in range(cfg.NCORE)]
    full = np.concatenate(outs, axis=0)[: cfg.N]
    return full, res


_BUILT = {}


def kernel(x, edge_index, w1, b1, w2, b2):
    inputs = dict(x=x, edge_index=edge_index, w1=w1, b1=b1, w2=w2, b2=b2)
    cfg = Cfg(N=50000, E=800000)
    meta, in_maps = make_host_data(cfg, inputs)
    key = (meta["NB0"], meta["NB1"])
    if key not in _BUILT:
        _BUILT[key] = build_nc(cfg, meta)
    out, _ = run(cfg, inputs, nc=_BUILT[key], meta=meta, in_maps=in_maps)
    return out.astype(np.float32)


# revision 20
# speedup vs baseline: 1.1737x; 1.1737x over previous
import math
import numpy as np
import ml_dtypes
from contextlib import ExitStack

import concourse.bass as bass
import concourse.tile as tile
from concourse import bacc, mybir
from concourse.bass_utils import run_bass_kernel_spmd

F32 = mybir.dt.float32
BF16 = mybir.dt.bfloat16
I16 = mybir.dt.int16
I32 = mybir.dt.int32


class Cfg:
    def __init__(self, N, E, F=128, HID=128, C=32, K=3, NCORE=8, CHUNK=None,
                 GBLK=32):
        self.N, self.E, self.F, self.HID, self.C, self.K = N, E, F, HID, C, K
        self.NCORE = NCORE
        if CHUNK is None:
            CHUNK = ((N + NCORE - 1) // NCORE + 127) // 128 * 128
        self.CHUNK = CHUNK
        self.NPAD = NCORE * CHUNK
        self.TILES = CHUNK // 128
        self.STRIPES = self.NPAD // 128
        self.GBLK = GBLK
        self.TA = (self.TILES + 1) // 2
        self.TB = self.TILES - self.TA
        self.RA = self.TA * 128
        self.RB = self.TB * 128
        self.HALF0 = NCORE * self.RA
        self.HALF1 = NCORE * self.RB
        assert self.HALF0 <= 32768 and self.HALF1 <= 32768, "int16 idx limit"

    def rowmap(self, n):
        c = n // self.CHUNK
        o = n % self.CHUNK
        return np.where(o < self.RA, c * self.RA + o,
                        self.HALF0 + c * self.RB + (o - self.RA))


def preprocess(cfg, edge_index, deg):
    c = cfg
    src, dst = edge_index[0].astype(np.int64), edge_index[1].astype(np.int64)
    owner = dst // c.CHUNK
    t_all = (dst % c.CHUNK) >> 7
    p_all = dst & 127
    srow = cfg.rowmap(src)
    h_all = (srow >= c.HALF0).astype(np.int64)

    key = (owner * c.TILES + t_all) * 2 + h_all
    order = np.lexsort((srow, key))
    key_s = key[order]
    srow_s = srow[order]
    p_s = p_all[order]
    ngrp = c.NCORE * c.TILES * 2
    counts = np.bincount(key_s, minlength=ngrp).reshape(c.NCORE, c.TILES, 2)
    starts = np.zeros(ngrp + 1, np.int64)
    np.cumsum(counts.reshape(-1), out=starts[1:])

    nblk = (counts + 127) // 128
    B = nblk.max(axis=0)
    B0, B1 = B[:, 0].copy(), B[:, 1].copy()
    NB0, NB1 = int(B0.sum()), int(B1.sum())
    TOTBLK = NB0 + NB1

    idx0 = np.zeros((c.NCORE, NB0 * 128), np.int16)
    idx1 = np.zeros((c.NCORE, NB1 * 128), np.int16)
    pv = np.full((c.NCORE, TOTBLK * 128), 128.0, np.float32)

    s0_off = np.concatenate([[0], np.cumsum(B0)])
    s1_off = np.concatenate([[0], np.cumsum(B1)])

    for core in range(c.NCORE):
        for t in range(c.TILES):
            for h, (idxa, soff) in enumerate(((idx0, s0_off), (idx1, s1_off))):
                g = (core * c.TILES + t) * 2 + h
                n = int(counts[core, t, h])
                a = int(starts[g])
                lidx = (srow_s[a : a + n] - h * c.HALF0).astype(np.int16)
                ps = p_s[a : a + n].astype(np.float32)
                base = int(soff[t]) * 128
                idxa[core, base : base + n] = lidx
                pbase = (0 if h == 0 else NB0 * 128) + base
                pv[core, pbase : pbase + n] = ps

    def wrap_idx(a):
        m = a.reshape(a.shape[0], -1, 16)
        m = np.swapaxes(m, 1, 2)
        return np.tile(m, (1, 8, 1)).copy()

    return dict(
        B0=B0, B1=B1, NB0=NB0, NB1=NB1, TOTBLK=TOTBLK,
        idx0=wrap_idx(idx0) if NB0 else np.zeros((c.NCORE, 128, 0), np.int16),
        idx1=wrap_idx(idx1) if NB1 else np.zeros((c.NCORE, 128, 0), np.int16),
        pv=pv.reshape(c.NCORE, TOTBLK, 128).transpose(0, 2, 1).copy(),
    )


def _calls(total, gblk):
    out = []
    b = 0
    while b < total:
        nb = min(gblk, total - b)
        out.append((b, nb))
        b += nb
    return out


def build_nc(cfg, meta):
    c = cfg
    B0, B1 = meta["B0"], meta["B1"]
    NB0, NB1, TOTBLK = meta["NB0"], meta["NB1"], meta["TOTBLK"]
    calls0 = _calls(NB0, c.GBLK)
    calls1 = _calls(NB1, c.GBLK)
    s0_off = np.concatenate([[0], np.cumsum(B0)])
    s1_off = np.concatenate([[0], np.cumsum(B1)])

    nc = bacc.Bacc(None, target_bir_lowering=False, num_swdge_queues=4)

    xchunk = nc.declare_dram_parameter("xchunk", [128, c.TILES, c.F], F32, isOutput=False)
    tia_d = nc.declare_dram_parameter("tia", [c.HALF0, c.F], BF16, isOutput=False)
    tib_d = nc.declare_dram_parameter("tib", [c.HALF1, c.F], BF16, isOutput=False)
    idx0_d = nc.declare_dram_parameter("idx0", [128, max(NB0 * 8, 1)], I16, isOutput=False)
    idx1_d = nc.declare_dram_parameter("idx1", [128, max(NB1 * 8, 1)], I16, isOutput=False)
    m_d = nc.declare_dram_parameter("m_in", [128, TOTBLK, 128], BF16, isOutput=False)
    discol_d = nc.declare_dram_parameter("discol", [128, c.TILES], F32, isOutput=False)
    w1_d = nc.declare_dram_parameter("w1", [c.K + 1, c.F, c.HID], F32, isOutput=False)
    b1_d = nc.declare_dram_parameter("b1", [c.HID, 1], F32, isOutput=False)
    w2_d = nc.declare_dram_parameter("w2", [c.K + 1, c.HID, c.C], F32, isOutput=False)
    b2_d = nc.declare_dram_parameter("b2", [c.C, 1], F32, isOutput=False)
    out_d = nc.declare_dram_parameter("out", [c.CHUNK, c.C], F32, isOutput=True)

    ta = [nc.dram_tensor(f"tab_a{i}", [c.HALF0, c.F], BF16, kind="Internal",
                         addr_space="Shared") for i in range(2)]
    tb = [nc.dram_tensor(f"tab_b{i}", [c.HALF1, c.F], BF16, kind="Internal",
                         addr_space="Shared") for i in range(2)]
    stage_a = nc.dram_tensor("stage_a", [c.RA, c.F], BF16, kind="Internal")
    stage_b = nc.dram_tensor("stage_b", [c.RB, c.F], BF16, kind="Internal")

    with tile.TileContext(nc) as tc, ExitStack() as ctx:
        sp = ctx.enter_context(tc.tile_pool(name="sp", bufs=1))
        gp0 = ctx.enter_context(tc.tile_pool(name="gp0", bufs=4))
        gp1 = ctx.enter_context(tc.tile_pool(name="gp1", bufs=4))
        mp = ctx.enter_context(tc.tile_pool(name="mp", bufs=3))
        wp = ctx.enter_context(tc.tile_pool(name="wp", bufs=2))
        ps_seg = ctx.enter_context(tc.tile_pool(name="ps_seg", bufs=4, space="PSUM"))
        ps_tr = ctx.enter_context(tc.tile_pool(name="ps_tr", bufs=2, space="PSUM"))
        ps_w = ctx.enter_context(tc.tile_pool(name="ps_w", bufs=2, space="PSUM"))

        idx0_t = sp.tile([128, max(NB0 * 8, 1)], I16)
        nc.sync.dma_start(idx0_t[:], idx0_d[:, :])
        idx1_t = sp.tile([128, max(NB1 * 8, 1)], I16)
        nc.sync.dma_start(idx1_t[:], idx1_d[:, :])
        discol_t = sp.tile([128, c.TILES], F32)
        nc.sync.dma_start(discol_t[:], discol_d[:, :])
        b1_t = sp.tile([c.HID, 1], F32)
        nc.sync.dma_start(b1_t[:], b1_d[:, :])
        b2_t = sp.tile([c.C, 1], F32)
        nc.sync.dma_start(b2_t[:], b2_d[:, :])

        w1_t, w2_t = [], []
        for k in range(c.K + 1):
            wf = wp.tile([c.F, c.HID], F32, tag="wload", name="wload")
            nc.sync.dma_start(wf[:], w1_d[k, :, :])
            wb = sp.tile([c.F, c.HID], BF16, tag=f"w1_{k}", name=f"w1_{k}")
            nc.vector.tensor_copy(wb[:], wf[:])
            w1_t.append(wb)
        for k in range(c.K + 1):
            wf = wp.tile([c.HID, c.C], F32, tag="wload2", name="wload2")
            nc.sync.dma_start(wf[:], w2_d[k, :, :])
            wb = sp.tile([c.HID, c.C], BF16, tag=f"w2_{k}", name=f"w2_{k}")
            nc.vector.tensor_copy(wb[:], wf[:])
            w2_t.append(wb)

        iota_i = sp.tile([128, 128], I32)
        nc.gpsimd.iota(iota_i[:], pattern=[[1, 128]], base=0, channel_multiplier=0)
        iota_bf = sp.tile([128, 128], BF16)
        nc.vector.tensor_copy(iota_bf[:], iota_i[:])
        iota_f = sp.tile([128, 128], F32)
        nc.vector.tensor_copy(iota_f[:], iota_i[:])
        iop_i = sp.tile([128, 1], I32)
        nc.gpsimd.iota(iop_i[:], pattern=[[1, 1]], base=0, channel_multiplier=1)
        iop_f = sp.tile([128, 1], F32)
        nc.vector.tensor_copy(iop_f[:], iop_i[:])
        ident_bf = sp.tile([128, 128], BF16)
        nc.vector.tensor_scalar(ident_bf[:], iota_bf[:], iop_f[:], None,
                                mybir.AluOpType.is_equal)
        ident_f = sp.tile([128, 128], F32)
        nc.vector.tensor_scalar(ident_f[:], iota_f[:], iop_f[:], None,
                                mybir.AluOpType.is_equal)

        stash = [sp.tile([128, c.CHUNK], BF16, tag=f"stash{k}", name=f"stash{k}")
                 for k in range(c.K + 1)]
        staging = sp.tile([128, c.TILES, c.F], BF16, tag="staging", name="staging")

        gfull_reg = nc.gpsimd.to_reg(c.GBLK * 128)

        for t in range(c.TILES):
            xt = wp.tile([128, c.F], F32, tag="xT", name="xT")
            nc.sync.dma_start(xt[:], xchunk[:, t, :])
            pt = ps_tr.tile([128, 128], F32, tag="ptr", name="ptr_f")
            nc.tensor.transpose(pt[:], xt[:], ident_f[:])
            nc.vector.tensor_copy(stash[0][:, t * 128 : (t + 1) * 128], pt[:])

        def ag_a(par_out):
            nc.sync.dma_start(
                stage_a[:, :].rearrange("(t p) f -> p t f", p=128),
                staging[:, 0 : c.TA, :])
            nc.gpsimd.collective_compute(
                "AllGather", mybir.AluOpType.bypass,
                replica_groups=[list(range(c.NCORE))],
                ins=[stage_a[:, :].opt()],
                outs=[ta[par_out][:, :].opt()],
            )

        def ag_b(par_out):
            nc.sync.dma_start(
                stage_b[:, :].rearrange("(t p) f -> p t f", p=128),
                staging[:, c.TA : c.TILES, :])
            nc.gpsimd.collective_compute(
                "AllGather", mybir.AluOpType.bypass,
                replica_groups=[list(range(c.NCORE))],
                ins=[stage_b[:, :].opt()],
                outs=[tb[par_out][:, :].opt()],
            )

        qctr = [0]

        def stream_slot(bufs, pos):
            for (b, nb, t_) in bufs:
                if b <= pos < b + nb:
                    return t_[:, pos - b, :]
            raise AssertionError(pos)

        def stage_tile(k, t):
            tr = slice(t * 128, (t + 1) * 128)
            pt = ps_tr.tile([128, 128], BF16, tag="ptr", name="ptr_b")
            nc.tensor.transpose(pt[:], stash[k][:, tr], ident_bf[:])
            nc.scalar.activation(staging[:, t, :], pt[:],
                                 mybir.ActivationFunctionType.Copy,
                                 scale=discol_t[:, t : t + 1])

        def hop(par, k, last, out_par=None):
            gatA, gatB, mA, mB = [], [], [], []

            srcA, srcB = (tia_d, tib_d) if par is None else (ta[par], tb[par])

            def gcall(half, b, nb):
                pool, idxt, tab, moff = ((gp0, idx0_t, srcA, 0) if half == 0
                                         else (gp1, idx1_t, srcB, NB0))
                tag = f"g{half}buf"
                gt = pool.tile([128, c.GBLK, 128], BF16, tag=tag, name=tag)
                nreg = nb * 128
                nc.gpsimd.dma_gather(gt[:, 0:nb, :], tab[:, :],
                                     idxt[:, b * 8 : (b + nb) * 8],
                                     nb * 128, nreg, c.F,
                                     single_packet=False,
                                     queue_num=qctr[0] % 4)
                qctr[0] += 1
                (gatA if half == 0 else gatB).append((b, nb, gt))
                mt = mp.tile([128, c.GBLK, 128], BF16, tag="mstream", name="mstream")
                nc.sync.dma_start(mt[:, 0:nb, :], m_d[:, moff + b : moff + b + nb, :])
                (mA if half == 0 else mB).append((b, nb, mt))

            FRONT0, FRONT1 = 3, 2
            seq = [(0, b, nb) for (b, nb) in calls0[:FRONT0]]
            seq += [(1, b, nb) for (b, nb) in calls1[:FRONT1]]
            rest0, rest1 = calls0[FRONT0:], calls1[FRONT1:]
            i0 = i1 = 0
            while i1 < len(rest1) or i0 < len(rest0):
                if i0 < len(rest0):
                    seq.append((0,) + rest0[i0]); i0 += 1
                if i1 < len(rest1):
                    seq.append((1,) + rest1[i1]); i1 += 1
            for (half, b, nb) in seq:
                gcall(half, b, nb)

            for t in range(c.TILES):
                nb0, nb1 = int(B0[t]), int(B1[t])
                nbt = nb0 + nb1
                tr = slice(t * 128, (t + 1) * 128)
                if nbt == 0:
                    nc.vector.memset(stash[k][:, tr], 0.0)
                else:
                    ps = ps_seg.tile([128, 128], F32, tag="seg", name="seg")
                    j = 0
                    for jj in range(nb0):
                        bsl = stream_slot(gatA, int(s0_off[t]) + jj)
                        msl = stream_slot(mA, int(s0_off[t]) + jj)
                        nc.tensor.matmul(ps[:], bsl, msl, start=(j == 0),
                                         stop=(j == nbt - 1))
                        j += 1
                    for jj in range(nb1):
                        bsl = stream_slot(gatB, int(s1_off[t]) + jj)
                        msl = stream_slot(mB, int(s1_off[t]) + jj)
                        nc.tensor.matmul(ps[:], bsl, msl, start=(j == 0),
                                         stop=(j == nbt - 1))
                        j += 1
                    nc.vector.tensor_copy(stash[k][:, tr], ps[:])
                if not last:
                    stage_tile(k, t)
                    if t == c.TA - 1:
                        ag_a(out_par)
            if not last:
                ag_b(out_par)

        def layer_end(layer):
            if layer == 1:
                fout, w_t = c.HID, w1_t
            else:
                fout, w_t = c.C, w2_t
            for t in range(c.TILES):
                tr = slice(t * 128, (t + 1) * 128)
                ps = ps_w.tile([fout, 128], F32, tag="wps", name="wps")
                for k in range(c.K + 1):
                    nc.tensor.matmul(ps[:], w_t[k][:], stash[k][:, tr],
                                     start=(k == 0), stop=(k == c.K))
                if layer == 1:
                    nc.scalar.activation(stash[0][:, tr], ps[:],
                                         mybir.ActivationFunctionType.Relu,
                                         bias=b1_t[:, 0:1])
                    stage_tile(0, t)
                    if t == c.TA - 1:
                        ag_a(0)
                else:
                    o2 = wp.tile([c.C, 128], F32, tag="o2T", name="o2T")
                    nc.vector.tensor_scalar(o2[:], ps[:], b2_t[:, 0:1], None,
                                            mybir.AluOpType.add)
                    pt2 = ps_tr.tile([128, c.C], F32, tag="ptr", name="ptr_o")
                    nc.tensor.transpose(pt2[:], o2[:], ident_f[0 : c.C, 0 : c.C])
                    ot = wp.tile([128, c.C], F32, tag="ofin", name="ofin")
                    nc.vector.tensor_copy(ot[:], pt2[:])
                    nc.sync.dma_start(out_d[t * 128 : (t + 1) * 128, :], ot[:])
            if layer == 1:
                ag_b(0)

        hop(None, 1, False, out_par=0)
        hop(0, 2, False, out_par=1)
        hop(1, 3, True)
        layer_end(1)
        hop(0, 1, False, out_par=1)
        hop(1, 2, False, out_par=0)
        hop(0, 3, True)
        layer_end(2)

    nc.finalize()
    return nc


def make_host_data(cfg, inputs):
    c = cfg
    x = np.asarray(inputs["x"], np.float32)
    ei = np.asarray(inputs["edge_index"])
    w1 = np.asarray(inputs["w1"], np.float32)
    b1 = np.asarray(inputs["b1"], np.float32)
    w2 = np.asarray(inputs["w2"], np.float32)
    b2 = np.asarray(inputs["b2"], np.float32)

    deg = np.bincount(ei[1], minlength=c.N).astype(np.float32)
    dis = np.where(deg > 0, np.maximum(deg, 1.0) ** -0.5, 0.0).astype(np.float32)
    dis_pad = np.zeros(c.NPAD, np.float32)
    dis_pad[: c.N] = dis

    meta = preprocess(c, ei, deg)

    xpad = np.zeros((c.NPAD, c.F), np.float32)
    xpad[: c.N] = x
    rows = np.asarray(c.rowmap(np.arange(c.NPAD)))
    g0 = (xpad * dis_pad[:, None]).astype(ml_dtypes.bfloat16)
    gtab = np.zeros_like(g0)
    gtab[rows] = g0
    tia = gtab[: c.HALF0].copy()
    tib = gtab[c.HALF0 :].copy()
    xchunk_all = xpad.reshape(c.NCORE, c.TILES, 128, c.F)

    B0, B1 = meta["B0"], meta["B1"]
    s0_off = np.concatenate([[0], np.cumsum(B0)])
    s1_off = np.concatenate([[0], np.cumsum(B1)])
    blk_tile = np.concatenate([
        np.repeat(np.arange(c.TILES), B0.astype(np.int64)),
        np.repeat(np.arange(c.TILES), B1.astype(np.int64)),
    ])

    in_maps = []
    for core in range(c.NCORE):
        r0, r1 = core * c.CHUNK, (core + 1) * c.CHUNK
        dchunk = dis_pad[r0:r1]
        pvc = meta["pv"][core]
        onehot = pvc[:, :, None] == np.arange(128, dtype=np.float32)[None, None, :]
        discols = dchunk.reshape(c.TILES, 128)[blk_tile]
        disb16 = discols.astype(ml_dtypes.bfloat16).astype(np.float32)
        m_in = (onehot * disb16[None, :, :]).astype(ml_dtypes.bfloat16)
        in_maps.append(dict(
            tia=tia, tib=tib,
            xchunk=xchunk_all[core].transpose(1, 0, 2).copy(),
            idx0=meta["idx0"][core],
            idx1=meta["idx1"][core],
            m_in=m_in,
            discol=dchunk.reshape(c.TILES, 128).T.copy(),
            w1=w1, b1=b1.reshape(c.HID, 1),
            w2=w2, b2=b2.reshape(c.C, 1),
        ))
    return meta, in_maps


def run(cfg, inputs, nc=None, meta=None, in_maps=None, trace=False):
    if meta is None or in_maps is None:
        meta, in_maps = make_host_data(cfg, inputs)
    if nc is None:
        nc = build_nc(cfg, meta)
    res = run_bass_kernel_spmd(nc, in_maps, list(range(cfg.NCORE)), trace=trace)
    outs = [res.results[i]["out"] for i in range(cfg.NCORE)]
    full = np.concatenate(outs, axis=0)[: cfg.N]
    return full, res


_BUILT = {}


def kernel(x, edge_index, w1, b1, w2, b2):
    inputs = dict(x=x, edge_index=edge_index, w1=w1, b1=b1, w2=w2, b2=b2)
    cfg = Cfg(N=50000, E=800000)
    meta, in_maps = make_host_data(cfg, inputs)
    key = (meta["NB0"], meta["NB1"])
    if key not in _BUILT:
        _BUILT[key] = build_nc(cfg, meta)
    out, _ = run(cfg, inputs, nc=_BUILT[key], meta=meta, in_maps=in_maps)
    return out.astype(np.float32)


# revision 23
# speedup vs baseline: 1.2907x; 1.0997x over previous
import math
import numpy as np
import ml_dtypes
from contextlib import ExitStack

import concourse.bass as bass
import concourse.tile as tile
from concourse import bacc, mybir
from concourse.bass_utils import run_bass_kernel_spmd

F32 = mybir.dt.float32
BF16 = mybir.dt.bfloat16
I16 = mybir.dt.int16
I32 = mybir.dt.int32


class Cfg:
    def __init__(self, N, E, F=128, HID=128, C=32, K=3, NCORE=8, CHUNK=None,
                 GBLK=32):
        self.N, self.E, self.F, self.HID, self.C, self.K = N, E, F, HID, C, K
        self.NCORE = NCORE
        if CHUNK is None:
            CHUNK = ((N + NCORE - 1) // NCORE + 127) // 128 * 128
        self.CHUNK = CHUNK
        self.NPAD = NCORE * CHUNK
        self.TILES = CHUNK // 128
        self.STRIPES = self.NPAD // 128
        self.GBLK = GBLK
        self.TA = min(32, self.TILES)
        self.TB = self.TILES - self.TA
        self.RA = self.TA * 128
        self.RB = self.TB * 128
        self.HALF0 = NCORE * self.RA
        self.HALF1 = NCORE * self.RB
        assert self.HALF0 <= 32768 and self.HALF1 <= 32768, "int16 idx limit"

    def rowmap(self, n):
        c = n // self.CHUNK
        o = n % self.CHUNK
        return np.where(o < self.RA, c * self.RA + o,
                        self.HALF0 + c * self.RB + (o - self.RA))


def preprocess(cfg, edge_index, deg):
    c = cfg
    src, dst = edge_index[0].astype(np.int64), edge_index[1].astype(np.int64)
    owner = dst // c.CHUNK
    t_all = (dst % c.CHUNK) >> 7
    p_all = dst & 127
    srow = cfg.rowmap(src)
    h_all = (srow >= c.HALF0).astype(np.int64)

    key = (owner * c.TILES + t_all) * 2 + h_all
    order = np.lexsort((srow, key))
    key_s = key[order]
    srow_s = srow[order]
    p_s = p_all[order]
    ngrp = c.NCORE * c.TILES * 2
    counts = np.bincount(key_s, minlength=ngrp).reshape(c.NCORE, c.TILES, 2)
    starts = np.zeros(ngrp + 1, np.int64)
    np.cumsum(counts.reshape(-1), out=starts[1:])

    nblk = (counts + 127) // 128
    B = nblk.max(axis=0)
    B0, B1 = B[:, 0].copy(), B[:, 1].copy()
    NB0, NB1 = int(B0.sum()), int(B1.sum())
    TOTBLK = NB0 + NB1

    idx0 = np.zeros((c.NCORE, NB0 * 128), np.int16)
    idx1 = np.zeros((c.NCORE, NB1 * 128), np.int16)
    pv = np.full((c.NCORE, TOTBLK * 128), 128.0, np.float32)

    s0_off = np.concatenate([[0], np.cumsum(B0)])
    s1_off = np.concatenate([[0], np.cumsum(B1)])

    for core in range(c.NCORE):
        for t in range(c.TILES):
            for h, (idxa, soff) in enumerate(((idx0, s0_off), (idx1, s1_off))):
                g = (core * c.TILES + t) * 2 + h
                n = int(counts[core, t, h])
                a = int(starts[g])
                lidx = (srow_s[a : a + n] - h * c.HALF0).astype(np.int16)
                ps = p_s[a : a + n].astype(np.float32)
                base = int(soff[t]) * 128
                idxa[core, base : base + n] = lidx
                pbase = (0 if h == 0 else NB0 * 128) + base
                pv[core, pbase : pbase + n] = ps

    def wrap_idx(a):
        m = a.reshape(a.shape[0], -1, 16)
        m = np.swapaxes(m, 1, 2)
        return np.tile(m, (1, 8, 1)).copy()

    return dict(
        B0=B0, B1=B1, NB0=NB0, NB1=NB1, TOTBLK=TOTBLK,
        idx0=wrap_idx(idx0) if NB0 else np.zeros((c.NCORE, 128, 0), np.int16),
        idx1=wrap_idx(idx1) if NB1 else np.zeros((c.NCORE, 128, 0), np.int16),
        pv=pv.reshape(c.NCORE, TOTBLK, 128).transpose(0, 2, 1).copy(),
    )


def _calls(total, gblk):
    out = []
    b = 0
    while b < total:
        nb = min(gblk, total - b)
        out.append((b, nb))
        b += nb
    return out


def build_nc(cfg, meta):
    c = cfg
    B0, B1 = meta["B0"], meta["B1"]
    NB0, NB1, TOTBLK = meta["NB0"], meta["NB1"], meta["TOTBLK"]
    calls0 = _calls(NB0, c.GBLK)
    calls1 = _calls(NB1, c.GBLK)
    s0_off = np.concatenate([[0], np.cumsum(B0)])
    s1_off = np.concatenate([[0], np.cumsum(B1)])

    nc = bacc.Bacc(None, target_bir_lowering=False, num_swdge_queues=4)

    xchunk = nc.declare_dram_parameter("xchunk", [128, c.TILES, c.F], F32, isOutput=False)
    tia_d = nc.declare_dram_parameter("tia", [c.HALF0, c.F], BF16, isOutput=False)
    tib_d = nc.declare_dram_parameter("tib", [c.HALF1, c.F], BF16, isOutput=False)
    idx0_d = nc.declare_dram_parameter("idx0", [128, max(NB0 * 8, 1)], I16, isOutput=False)
    idx1_d = nc.declare_dram_parameter("idx1", [128, max(NB1 * 8, 1)], I16, isOutput=False)
    m_d = nc.declare_dram_parameter("m_in", [128, TOTBLK, 128], BF16, isOutput=False)
    discol_d = nc.declare_dram_parameter("discol", [128, c.TILES], F32, isOutput=False)
    w1_d = nc.declare_dram_parameter("w1", [c.K + 1, c.F, c.HID], F32, isOutput=False)
    b1_d = nc.declare_dram_parameter("b1", [c.HID, 1], F32, isOutput=False)
    w2_d = nc.declare_dram_parameter("w2", [c.K + 1, c.HID, c.C], F32, isOutput=False)
    b2_d = nc.declare_dram_parameter("b2", [c.C, 1], F32, isOutput=False)
    out_d = nc.declare_dram_parameter("out", [c.CHUNK, c.C], F32, isOutput=True)

    ta = [nc.dram_tensor(f"tab_a{i}", [c.HALF0, c.F], BF16, kind="Internal",
                         addr_space="Shared") for i in range(2)]
    tb = [nc.dram_tensor(f"tab_b{i}", [c.HALF1, c.F], BF16, kind="Internal",
                         addr_space="Shared") for i in range(2)]
    stage_a = nc.dram_tensor("stage_a", [c.RA, c.F], BF16, kind="Internal")
    stage_b = nc.dram_tensor("stage_b", [c.RB, c.F], BF16, kind="Internal")

    with tile.TileContext(nc) as tc, ExitStack() as ctx:
        sp = ctx.enter_context(tc.tile_pool(name="sp", bufs=1))
        gp0 = ctx.enter_context(tc.tile_pool(name="gp0", bufs=5))
        gp1 = ctx.enter_context(tc.tile_pool(name="gp1", bufs=5))
        mp = ctx.enter_context(tc.tile_pool(name="mp", bufs=3))
        wp = ctx.enter_context(tc.tile_pool(name="wp", bufs=2))
        ps_seg = ctx.enter_context(tc.tile_pool(name="ps_seg", bufs=4, space="PSUM"))
        ps_tr = ctx.enter_context(tc.tile_pool(name="ps_tr", bufs=2, space="PSUM"))
        ps_w = ctx.enter_context(tc.tile_pool(name="ps_w", bufs=2, space="PSUM"))

        idx0_t = sp.tile([128, max(NB0 * 8, 1)], I16)
        nc.sync.dma_start(idx0_t[:], idx0_d[:, :])
        idx1_t = sp.tile([128, max(NB1 * 8, 1)], I16)
        nc.sync.dma_start(idx1_t[:], idx1_d[:, :])
        discol_t = sp.tile([128, c.TILES], F32)
        nc.sync.dma_start(discol_t[:], discol_d[:, :])
        b1_t = sp.tile([c.HID, 1], F32)
        nc.sync.dma_start(b1_t[:], b1_d[:, :])
        b2_t = sp.tile([c.C, 1], F32)
        nc.sync.dma_start(b2_t[:], b2_d[:, :])

        w1_t, w2_t = [], []
        for k in range(c.K + 1):
            wf = wp.tile([c.F, c.HID], F32, tag="wload", name="wload")
            nc.sync.dma_start(wf[:], w1_d[k, :, :])
            wb = sp.tile([c.F, c.HID], BF16, tag=f"w1_{k}", name=f"w1_{k}")
            nc.vector.tensor_copy(wb[:], wf[:])
            w1_t.append(wb)
        for k in range(c.K + 1):
            wf = wp.tile([c.HID, c.C], F32, tag="wload2", name="wload2")
            nc.sync.dma_start(wf[:], w2_d[k, :, :])
            wb = sp.tile([c.HID, c.C], BF16, tag=f"w2_{k}", name=f"w2_{k}")
            nc.vector.tensor_copy(wb[:], wf[:])
            w2_t.append(wb)

        iota_i = sp.tile([128, 128], I32)
        nc.gpsimd.iota(iota_i[:], pattern=[[1, 128]], base=0, channel_multiplier=0)
        iota_bf = sp.tile([128, 128], BF16)
        nc.vector.tensor_copy(iota_bf[:], iota_i[:])
        iota_f = sp.tile([128, 128], F32)
        nc.vector.tensor_copy(iota_f[:], iota_i[:])
        iop_i = sp.tile([128, 1], I32)
        nc.gpsimd.iota(iop_i[:], pattern=[[1, 1]], base=0, channel_multiplier=1)
        iop_f = sp.tile([128, 1], F32)
        nc.vector.tensor_copy(iop_f[:], iop_i[:])
        ident_bf = sp.tile([128, 128], BF16)
        nc.vector.tensor_scalar(ident_bf[:], iota_bf[:], iop_f[:], None,
                                mybir.AluOpType.is_equal)
        ident_f = sp.tile([128, 128], F32)
        nc.vector.tensor_scalar(ident_f[:], iota_f[:], iop_f[:], None,
                                mybir.AluOpType.is_equal)

        stash = [sp.tile([128, c.CHUNK], BF16, tag=f"stash{k}", name=f"stash{k}")
                 for k in range(c.K + 1)]
        staging = sp.tile([128, c.TILES, c.F], BF16, tag="staging", name="staging")

        gfull_reg = nc.gpsimd.to_reg(c.GBLK * 128)

        for t in range(c.TILES):
            xt = wp.tile([128, c.F], F32, tag="xT", name="xT")
            nc.sync.dma_start(xt[:], xchunk[:, t, :])
            pt = ps_tr.tile([128, 128], F32, tag="ptr", name="ptr_f")
            nc.tensor.transpose(pt[:], xt[:], ident_f[:])
            nc.vector.tensor_copy(stash[0][:, t * 128 : (t + 1) * 128], pt[:])

        def ag_a(par_out):
            nc.sync.dma_start(
                stage_a[:, :].rearrange("(t p) f -> p t f", p=128),
                staging[:, 0 : c.TA, :])
            nc.gpsimd.collective_compute(
                "AllGather", mybir.AluOpType.bypass,
                replica_groups=[list(range(c.NCORE))],
                ins=[stage_a[:, :].opt()],
                outs=[ta[par_out][:, :].opt()],
            )

        def ag_b(par_out):
            nc.sync.dma_start(
                stage_b[:, :].rearrange("(t p) f -> p t f", p=128),
                staging[:, c.TA : c.TILES, :])
            nc.gpsimd.collective_compute(
                "AllGather", mybir.AluOpType.bypass,
                replica_groups=[list(range(c.NCORE))],
                ins=[stage_b[:, :].opt()],
                outs=[tb[par_out][:, :].opt()],
            )

        qctr = [0]

        def stream_slot(bufs, pos):
            for (b, nb, t_) in bufs:
                if b <= pos < b + nb:
                    return t_[:, pos - b, :]
            raise AssertionError(pos)

        def stage_tile(k, t):
            tr = slice(t * 128, (t + 1) * 128)
            pt = ps_tr.tile([128, 128], BF16, tag="ptr", name="ptr_b")
            nc.tensor.transpose(pt[:], stash[k][:, tr], ident_bf[:])
            nc.scalar.activation(staging[:, t, :], pt[:],
                                 mybir.ActivationFunctionType.Copy,
                                 scale=discol_t[:, t : t + 1])

        def hop(par, k, last, out_par=None):
            gatA, gatB, mA, mB = [], [], [], []

            srcA, srcB = (tia_d, tib_d) if par is None else (ta[par], tb[par])

            def gcall(half, b, nb):
                pool, idxt, tab, moff = ((gp0, idx0_t, srcA, 0) if half == 0
                                         else (gp1, idx1_t, srcB, NB0))
                tag = f"g{half}buf"
                gt = pool.tile([128, c.GBLK, 128], BF16, tag=tag, name=tag)
                nreg = nb * 128
                nc.gpsimd.dma_gather(gt[:, 0:nb, :], tab[:, :],
                                     idxt[:, b * 8 : (b + nb) * 8],
                                     nb * 128, nreg, c.F,
                                     single_packet=False,
                                     queue_num=qctr[0] % 4)
                qctr[0] += 1
                (gatA if half == 0 else gatB).append((b, nb, gt))
                mt = mp.tile([128, c.GBLK, 128], BF16, tag="mstream", name="mstream")
                nc.sync.dma_start(mt[:, 0:nb, :], m_d[:, moff + b : moff + b + nb, :])
                (mA if half == 0 else mB).append((b, nb, mt))

            def start_tile(soff, b):
                return int(np.searchsorted(soff, b, side="right") - 1)

            items = [(start_tile(s0_off, b), 0, b, nb) for (b, nb) in calls0]
            items += [(start_tile(s1_off, b), 1, b, nb) for (b, nb) in calls1]
            items.sort(key=lambda x: (x[0], x[1]))
            for (_, half, b, nb) in items:
                gcall(half, b, nb)

            for t in range(c.TILES):
                nb0, nb1 = int(B0[t]), int(B1[t])
                nbt = nb0 + nb1
                tr = slice(t * 128, (t + 1) * 128)
                if nbt == 0:
                    nc.vector.memset(stash[k][:, tr], 0.0)
                else:
                    ps = ps_seg.tile([128, 128], F32, tag="seg", name="seg")
                    j = 0
                    for jj in range(nb0):
                        bsl = stream_slot(gatA, int(s0_off[t]) + jj)
                        msl = stream_slot(mA, int(s0_off[t]) + jj)
                        nc.tensor.matmul(ps[:], bsl, msl, start=(j == 0),
                                         stop=(j == nbt - 1))
                        j += 1
                    for jj in range(nb1):
                        bsl = stream_slot(gatB, int(s1_off[t]) + jj)
                        msl = stream_slot(mB, int(s1_off[t]) + jj)
                        nc.tensor.matmul(ps[:], bsl, msl, start=(j == 0),
                                         stop=(j == nbt - 1))
                        j += 1
                    nc.vector.tensor_copy(stash[k][:, tr], ps[:])
                if not last:
                    stage_tile(k, t)
                    if t == c.TA - 1:
                        ag_a(out_par)
            if not last:
                ag_b(out_par)

        def layer_end(layer):
            if layer == 1:
                fout, w_t = c.HID, w1_t
            else:
                fout, w_t = c.C, w2_t
            for t in range(c.TILES):
                tr = slice(t * 128, (t + 1) * 128)
                ps = ps_w.tile([fout, 128], F32, tag="wps", name="wps")
                for k in range(c.K + 1):
                    nc.tensor.matmul(ps[:], w_t[k][:], stash[k][:, tr],
                                     start=(k == 0), stop=(k == c.K))
                if layer == 1:
                    nc.scalar.activation(stash[0][:, tr], ps[:],
                                         mybir.ActivationFunctionType.Relu,
                                         bias=b1_t[:, 0:1])
                    stage_tile(0, t)
                    if t == c.TA - 1:
                        ag_a(0)
                else:
                    o2 = wp.tile([c.C, 128], F32, tag="o2T", name="o2T")
                    nc.vector.tensor_scalar(o2[:], ps[:], b2_t[:, 0:1], None,
                                            mybir.AluOpType.add)
                    pt2 = ps_tr.tile([128, c.C], F32, tag="ptr", name="ptr_o")
                    nc.tensor.transpose(pt2[:], o2[:], ident_f[0 : c.C, 0 : c.C])
                    ot = wp.tile([128, c.C], F32, tag="ofin", name="ofin")
                    nc.vector.tensor_copy(ot[:], pt2[:])
                    nc.sync.dma_start(out_d[t * 128 : (t + 1) * 128, :], ot[:])
            if layer == 1:
                ag_b(0)

        hop(None, 1, False, out_par=0)
        hop(0, 2, False, out_par=1)
        hop(1, 3, True)
        layer_end(1)
        hop(0, 1, False, out_par=1)
        hop(1, 2, False, out_par=0)
        hop(0, 3, True)
        layer_end(2)

    nc.finalize()
    return nc


def make_host_data(cfg, inputs):
    c = cfg
    x = np.asarray(inputs["x"], np.float32)
    ei = np.asarray(inputs["edge_index"])
    w1 = np.asarray(inputs["w1"], np.float32)
    b1 = np.asarray(inputs["b1"], np.float32)
    w2 = np.asarray(inputs["w2"], np.float32)
    b2 = np.asarray(inputs["b2"], np.float32)

    deg = np.bincount(ei[1], minlength=c.N).astype(np.float32)
    dis = np.where(deg > 0, np.maximum(deg, 1.0) ** -0.5, 0.0).astype(np.float32)
    dis_pad = np.zeros(c.NPAD, np.float32)
    dis_pad[: c.N] = dis

    meta = preprocess(c, ei, deg)

    xpad = np.zeros((c.NPAD, c.F), np.float32)
    xpad[: c.N] = x
    rows = np.asarray(c.rowmap(np.arange(c.NPAD)))
    g0 = (xpad * dis_pad[:, None]).astype(ml_dtypes.bfloat16)
    gtab = np.zeros_like(g0)
    gtab[rows] = g0
    tia = gtab[: c.HALF0].copy()
    tib = gtab[c.HALF0 :].copy()
    xchunk_all = xpad.reshape(c.NCORE, c.TILES, 128, c.F)

    B0, B1 = meta["B0"], meta["B1"]
    s0_off = np.concatenate([[0], np.cumsum(B0)])
    s1_off = np.concatenate([[0], np.cumsum(B1)])
    blk_tile = np.concatenate([
        np.repeat(np.arange(c.TILES), B0.astype(np.int64)),
        np.repeat(np.arange(c.TILES), B1.astype(np.int64)),
    ])

    in_maps = []
    for core in range(c.NCORE):
        r0, r1 = core * c.CHUNK, (core + 1) * c.CHUNK
        dchunk = dis_pad[r0:r1]
        pvc = meta["pv"][core]
        onehot = pvc[:, :, None] == np.arange(128, dtype=np.float32)[None, None, :]
        discols = dchunk.reshape(c.TILES, 128)[blk_tile]
        disb16 = discols.astype(ml_dtypes.bfloat16).astype(np.float32)
        m_in = (onehot * disb16[None, :, :]).astype(ml_dtypes.bfloat16)
        in_maps.append(dict(
            tia=tia, tib=tib,
            xchunk=xchunk_all[core].transpose(1, 0, 2).copy(),
            idx0=meta["idx0"][core],
            idx1=meta["idx1"][core],
            m_in=m_in,
            discol=dchunk.reshape(c.TILES, 128).T.copy(),
            w1=w1, b1=b1.reshape(c.HID, 1),
            w2=w2, b2=b2.reshape(c.C, 1),
        ))
    return meta, in_maps


def run(cfg, inputs, nc=None, meta=None, in_maps=None, trace=False):
    if meta is None or in_maps is None:
        meta, in_maps = make_host_data(cfg, inputs)
    if nc is None:
        nc = build_nc(cfg, meta)
    res = run_bass_kernel_spmd(nc, in_maps, list(range(cfg.NCORE)), trace=trace)
    outs = [res.results[i]["out"] for i in range(cfg.NCORE)]
    full = np.concatenate(outs, axis=0)[: cfg.N]
    return full, res


_BUILT = {}


def kernel(x, edge_index, w1, b1, w2, b2):
    inputs = dict(x=x, edge_index=edge_index, w1=w1, b1=b1, w2=w2, b2=b2)
    cfg = Cfg(N=50000, E=800000)
    meta, in_maps = make_host_data(cfg, inputs)
    key = (meta["NB0"], meta["NB1"])
    if key not in _BUILT:
        _BUILT[key] = build_nc(cfg, meta)
    out, _ = run(cfg, inputs, nc=_BUILT[key], meta=meta, in_maps=in_maps)
    return out.astype(np.float32)
